# revision 46
# baseline (speedup 1.0000x reference)
"""FFTBlock (attention + conv-FFN transformer block) on 8 Trainium2 NeuronCores.

Data-parallel over batch: 16 batch items -> 2 per core. Each core runs the
full block (MHA + LN + conv1d-FFN + LN) on its 2 batch items.

Layout strategy (per batch item, per core):
  - Host pre-transposes x -> xT [D,S] (bf16) so QKV projections contract D on
    partitions; host pre-transposes mask -> maskT [S_k,S_q] (uint8).
  - Q,K produced transposed (QT/KT [DH,S]); V natural [S,DH] with a ones
    column appended so the A@V matmul also produces softmax denominators.
  - Scores computed transposed (scoresT [S_k,S_q]); softmax is exp-only
    (no max subtraction needed: scores are O(1); masked lanes underflow to 0
    exactly, matching the reference), denominator comes from the ones column.
  - Out-projection consumes the normalized O-transposed chunks directly and
    lands attention output in natural [S,D] layout for a free-dim layernorm.
    bo is folded into Wo via a ones row in the head-0 second chunk.
  - LN1 output is stored UN-affined (g1/beta1 folded into Wc1/bc1 with a
    -beta1/g1 padding value for 'SAME' edges); the conv2 epilogue re-applies
    g1 and (beta1+bc2) to the residual.
  - conv1 produces hT [F,S] (weights stationary), conv2 consumes hT slices as
    stationary operands and lands natural [S,D] for the second layernorm.
  - Both convs implement K=9 'same' padding via shifted slices of a
    zero-padded S axis (4+S+4 columns).
  - All weights are host-pre-arranged into their final SBUF layouts so every
    weight DMA is a dense contiguous copy.
  - ~36 dummy matmuls at t=0 pre-warm the PE HAM clock gate during input DMA.
"""

import sys

sys.path.insert(0, "/opt/trn_rl_repo")

import math
from contextlib import ExitStack

import ml_dtypes
import numpy as np

import concourse.bass as bass
import concourse.mybir as mybir
import concourse.tile as tile
from concourse import bacc
from concourse.bass_utils import run_bass_kernel_spmd
from concourse.masks import make_identity

BF16 = mybir.dt.bfloat16
F32 = mybir.dt.float32
FP8 = mybir.dt.float8e4
DR = mybir.MatmulPerfMode.DoubleRow
AF = mybir.ActivationFunctionType
ALU = mybir.AluOpType

B, S, D, H, DH, F, K = 16, 1024, 384, 2, 192, 1536, 9
NCORES = 8
NB = B // NCORES  # batch items per core
EPS = 1e-5
ISCALE = 1.0 / math.sqrt(D)  # NOTE: reference scales by sqrt(d_model)
# fp8 attention pre-scales (keep e4m3 operands in its sweet range)
QKV_SC = 64.0  # Wq/Wk/Wv/bqk/bv scale; Q,K,V carried as 64x
WO_SC = 32.0  # Wo scale; out-proj PSUM = 64*32=2048x attn; xn pre-scaled 2048x
RES_SC = QKV_SC * WO_SC
SP = S + 8  # padded sequence length (4 left, 4 right)
DC = D // 128  # 3 d-chunks
FT = F // 128  # 12 filter tiles
ST = S // 128  # 8 seq tiles of 128
SQ = S // 512  # 2 seq chunks of 512

_CACHE = {}


def _bcast(ap, p=128):
    return bass.AP(tensor=ap.tensor, offset=ap.offset, ap=[[0, p]] + list(ap.ap))


def _emit(nc):
    # ---- DRAM I/O (all weights pre-arranged host-side to final SBUF layout) ----
    xT_d = nc.dram_tensor("xT", [NB, DC, 128, S], FP8, kind="ExternalInput")
    xn_d = nc.dram_tensor("xn", [NB, ST, 128, D], F32, kind="ExternalInput")
    mT_d = nc.dram_tensor("mT", [NB, ST, 128, S], FP8, kind="ExternalInput")
    wq_d = nc.dram_tensor("wq", [128, H, DC, DH], FP8, kind="ExternalInput")
    wk_d = nc.dram_tensor("wk", [128, H, DC, DH], FP8, kind="ExternalInput")
    wv_d = nc.dram_tensor("wv", [128, H, DC, DH], FP8, kind="ExternalInput")
    wo_d = nc.dram_tensor("wo", [128, 4, D], FP8, kind="ExternalInput")
    wc1_d = nc.dram_tensor("wc1", [FT, 128, K, DC, 128], BF16, kind="ExternalInput")
    wc2_d = nc.dram_tensor("wc2", [K, 128, FT, D], BF16, kind="ExternalInput")
    bqk_d = nc.dram_tensor("bqk", [128, 2, H, 2], F32, kind="ExternalInput")
    bvr_d = nc.dram_tensor("bvr", [H, 208], FP8, kind="ExternalInput")
    bc1_d = nc.dram_tensor("bc1t", [128, FT], F32, kind="ExternalInput")
    cb_d = nc.dram_tensor("cb", [D], F32, kind="ExternalInput")
    padv_d = nc.dram_tensor("padv", [DC, 128, 4], BF16, kind="ExternalInput")
    g1_d = nc.dram_tensor("g1", [D], F32, kind="ExternalInput")
    g2_d = nc.dram_tensor("g2", [D], F32, kind="ExternalInput")
    be2_d = nc.dram_tensor("be2", [D], F32, kind="ExternalInput")
    y_d = nc.dram_tensor("y", [NB, ST, 128, D], F32, kind="ExternalOutput")

    with tile.TileContext(nc) as tc:
        _body(nc, tc, locals())
    nc.finalize()
    return nc


def _body(nc, tc, d):
    xT_d, xn_d, mT_d = d["xT_d"], d["xn_d"], d["mT_d"]
    wq_d, wk_d, wv_d, wo_d = d["wq_d"], d["wk_d"], d["wv_d"], d["wo_d"]
    wc1_d, wc2_d = d["wc1_d"], d["wc2_d"]
    bqk_d, bvr_d, bc1_d, cb_d, padv_d = (
        d["bqk_d"], d["bvr_d"], d["bc1_d"], d["cb_d"], d["padv_d"],
    )
    g1_d, g2_d, be2_d, y_d = d["g1_d"], d["g2_d"], d["be2_d"], d["y_d"]

    with ExitStack() as ctx:
        const = ctx.enter_context(tc.tile_pool(name="const", bufs=1))
        persist = ctx.enter_context(tc.tile_pool(name="persist", bufs=1))

        # ---- critical-path weights first (dense contiguous DMAs) ----
        wq_sb = const.tile([128, H, DC, DH], FP8, tag="wq")
        nc.sync.dma_start(wq_sb[:], wq_d[:])
        bqk_sb = const.tile([128, 2, H, 2], F32, tag="bqk")
        nc.sync.dma_start(bqk_sb[:], bqk_d[:])

        # warm-up operand for HAM pre-warming
        warm_sb = const.tile([128, 128], BF16, tag="warm")
        nc.gpsimd.memset(warm_sb[:], 0.0)

        # ---- phased execution ----
        # P1: attention(b0) + qkv(b1) filler   P2: attention(b1) || conv1(b0)
        # P3: conv1(b1) || [w2 load + conv2(b0)]   P4: conv2(b1)
        with ExitStack() as octx:
            qkvp = octx.enter_context(tc.tile_pool(name="qkvp", bufs=1))
            xT_sb = [
                qkvp.tile([128, DC, S], FP8, name=f"xT{b}", tag=f"xT{b}")
                for b in range(NB)
            ]
            # b0's x first (unblocks Q/K h0 ASAP), then remaining weights
            for dc in range(DC):
                nc.sync.dma_start(xT_sb[0][:, dc, :], xT_d[0, dc])
            wk_sb = const.tile([128, H, DC, DH], FP8, tag="wk")
            nc.sync.dma_start(wk_sb[:], wk_d[:])
            wv_sb = const.tile([128, H, DC, DH], FP8, tag="wv")
            nc.sync.dma_start(wv_sb[:], wv_d[:])
            bvr_sb = const.tile([1, H, 208], FP8, tag="bvr")
            nc.sync.dma_start(bvr_sb[0:1], bvr_d[:])
            ones1 = const.tile([1, 128], FP8, tag="ones1")
            nc.gpsimd.memset(ones1[:], 1.0)
            for dc in range(DC):
                nc.sync.dma_start(xT_sb[1][:, dc, :], xT_d[1, dc])

            # remaining constants (off the critical path)
            wo_sb = const.tile([128, 4, D], FP8, tag="wo")
            nc.sync.dma_start(wo_sb[:], wo_d[:])
            ident = const.tile([128, 128], BF16, tag="ident")
            make_identity(nc, ident[:])
            identf = const.tile([128, 128], F32, tag="identf")
            make_identity(nc, identf[:])
            bc1_sb = const.tile([128, FT], F32, tag="bc1")
            nc.sync.dma_start(bc1_sb[:], bc1_d[:])
            cb_sb = const.tile([128, D], F32, tag="cb")
            nc.sync.dma_start(cb_sb[:], _bcast(cb_d[:]))
            g1_sb = const.tile([128, D], F32, tag="g1")
            nc.sync.dma_start(g1_sb[:], _bcast(g1_d[:]))
            g2_sb = const.tile([128, D], F32, tag="g2")
            nc.sync.dma_start(g2_sb[:], _bcast(g2_d[:]))
            be2_sb = const.tile([128, D], F32, tag="be2")
            nc.sync.dma_start(be2_sb[:], _bcast(be2_d[:]))
            eps_sb = const.tile([128, 1], F32, tag="eps")
            nc.vector.memset(eps_sb[:], EPS)

            x1T = persist.tile([128, NB, DC, SP], BF16, tag="x1T")
            x1n = persist.tile([128, NB, ST, D], BF16, tag="x1n")
            for b in range(NB):
                for dc in range(DC):
                    # 'SAME'-edge padding value -beta1/g1 (cancels the folded
                    # beta1 bias for taps that fall outside the sequence)
                    nc.sync.dma_start(x1T[:, b, dc, 0:4], padv_d[dc])
                    nc.sync.dma_start(x1T[:, b, dc, 4 + S : SP], padv_d[dc])

            QT, KT, VV, ON = {}, {}, {}, {}

            def weave_w(a, b, wts):
                # place filler units from b after a-units per-unit quota wts
                acc = 0.0
                ib = 0
                for ua, w in zip(a, wts):
                    ua()
                    acc += w
                    while ib < len(b) and acc >= 1.0:
                        b[ib]()
                        ib += 1
                        acc -= 1.0
                while ib < len(b):
                    b[ib]()
                    ib += 1

            def qkv_units(b, psA):
                units = []
                for h in range(H):
                    qt = qkvp.tile([128, 2, S], FP8, name=f"qt{b}{h}", tag=f"qt{b}{h}")
                    kt = qkvp.tile([128, 2, S], FP8, name=f"kt{b}{h}", tag=f"kt{b}{h}")
                    vv = qkvp.tile([128, ST, 208], FP8, name=f"vv{b}{h}", tag=f"vv{b}{h}")
                    QT[b, h], KT[b, h], VV[b, h] = qt, kt, vv
                    # zero the unused dh rows so DoubleRow pair-reads see 0
                    nc.gpsimd.memset(qt[64:128, 1, :], 0.0)
                    nc.gpsimd.memset(kt[64:128, 1, :], 0.0)
                    for wsb, bi, dst in ((wq_sb, 0, qt), (wk_sb, 1, kt)):
                        for mc, (m0, msz) in enumerate(((0, 128), (128, 64))):
                            for qc in range(SQ):
                                def u(b=b, h=h, wsb=wsb, bi=bi, dst=dst, m0=m0, msz=msz, mc=mc, qc=qc):
                                    qs = slice(qc * 512, qc * 512 + 512)
                                    ps = psA.tile([128, 512], F32, name="psqk", tag="p512")
                                    nc.tensor.matmul(
                                        ps[:msz, :],
                                        lhsT=wsb[:, h, 0:2, m0 : m0 + msz],
                                        rhs=xT_sb[b][:, 0:2, qs],
                                        start=True,
                                        stop=False,
                                        perf_mode=DR,
                                    )
                                    nc.tensor.matmul(
                                        ps[:msz, :],
                                        lhsT=wsb[:, h, 2, m0 : m0 + msz],
                                        rhs=xT_sb[b][:, 2, qs],
                                        start=False,
                                        stop=True,
                                    )
                                    nc.scalar.activation(
                                        out=dst[:msz, mc, qc * 512 : qc * 512 + 512],
                                        in_=ps[:msz, :],
                                        func=AF.Identity,
                                        bias=bqk_sb[:msz, bi, h, mc : mc + 1],
                                        scale=1.0,
                                    )
                                units.append(u)
                    for st in range(ST):
                        def u(b=b, h=h, vv=vv, st=st):
                            ss = slice(st * 128, st * 128 + 128)
                            ps = psA.tile([128, 512], F32, name="psv", tag="p512")
                            nc.tensor.matmul(
                                ps[:, :DH],
                                lhsT=xT_sb[b][:, 0:2, ss],
                                rhs=wv_sb[:, h, 0:2, :],
                                start=True,
                                stop=False,
                                perf_mode=DR,
                            )
                            nc.tensor.matmul(
                                ps[:, :DH],
                                lhsT=xT_sb[b][:, 2, ss],
                                rhs=wv_sb[:, h, 2, :],
                                start=False,
                                stop=False,
                            )
                            # rank-1 row adds bv AND writes the ones column
                            nc.tensor.matmul(
                                ps[:, 0 : DH + 1],
                                lhsT=ones1[:],
                                rhs=bvr_sb[0:1, h, 0 : DH + 1],
                                start=False,
                                stop=True,
                            )
                            nc.scalar.copy(
                                out=vv[:, st, 0 : DH + 1], in_=ps[:, 0 : DH + 1]
                            )
                        units.append(u)
                return units

            def attn_units(b, expp, mskp, smal, lnp, psA, psB, psC, attn):
                units, kinds = [], []
                expTs = {}
                for h in range(H):
                    expT = expp.tile([128, ST, S], FP8, name=f"expT{h}", tag="expT")
                    expTs[h] = expT
                    for kc in range(ST):
                        def u(b=b, h=h, expT=expT, kc=kc):
                            qt, kt = QT[b, h], KT[b, h]
                            mtile = mskp.tile([128, 1024], FP8, name="mt", tag="mt")
                            nc.sync.dma_start(mtile[:], mT_d[b, kc])
                            for qc in range(SQ):
                                qs = slice(qc * 512, qc * 512 + 512)
                                ps = psB.tile([128, 512], F32, name="pssc", tag="sc")
                                nc.tensor.matmul(
                                    ps[:],
                                    lhsT=kt[:, :, kc * 128 : kc * 128 + 128],
                                    rhs=qt[:, :, qs],
                                    start=True,
                                    stop=True,
                                    perf_mode=DR,
                                )
                                nc.scalar.activation(
                                    out=expT[:, kc, qs], in_=ps[:], func=AF.Exp,
                                    scale=ISCALE / (QKV_SC * QKV_SC),
                                )
                                nc.vector.tensor_mul(
                                    out=expT[:, kc, qs], in0=expT[:, kc, qs],
                                    in1=mtile[:, qs],
                                )
                        units.append(u)
                        kinds.append("sc")
                for h in range(H):
                    onrm = attn.tile([128, 2, S], FP8, name=f"on{b}{h}", tag=f"on{b}{h}")
                    ON[b, h] = onrm
                    nc.gpsimd.memset(onrm[64:128, 1, :], 0.0)
                    if h == 0:
                        # ones row (row 64 of chunk 1) -> adds bo via Wo's bo row
                        nc.gpsimd.memset(onrm[64:65, 1, :], 1.0)
                # alternate heads so AV matmul bursts overlap normalize chains
                for qc in range(SQ):
                    for h in range(H):
                        def u(b=b, h=h, qc=qc):
                            expT, onrm = expTs[h], ON[b, h]
                            vv = VV[b, h]
                            qs = slice(qc * 512, qc * 512 + 512)
                            ps0 = psC.tile([128, 512], F32, name="ps0", tag="ot")
                            ps1 = psC.tile([128, 512], F32, name="ps1", tag="ot")
                            for kc in range(0, ST, 2):
                                nc.tensor.matmul(
                                    ps0[:],
                                    lhsT=vv[:, kc : kc + 2, 0:128],
                                    rhs=expT[:, kc : kc + 2, qs],
                                    start=(kc == 0),
                                    stop=(kc == ST - 2),
                                    perf_mode=DR,
                                )
                                nc.tensor.matmul(
                                    ps1[:65, :],
                                    lhsT=vv[:, kc : kc + 2, 128 : DH + 1],
                                    rhs=expT[:, kc : kc + 2, qs],
                                    start=(kc == 0),
                                    stop=(kc == ST - 2),
                                    perf_mode=DR,
                                )
                            rc = smal.tile([1, 512], F32, tag="rc")
                            nc.scalar.copy(out=rc[:], in_=ps1[64:65, :])
                            rb = smal.tile([128, 512], F32, tag="rb")
                            nc.gpsimd.partition_broadcast(rb[:], rc[:])
                            nc.vector.reciprocal(rb[:], rb[:])
                            nc.vector.tensor_mul(out=onrm[:, 0, qs], in0=ps0[:], in1=rb[:])
                            nc.vector.tensor_mul(
                                out=onrm[:64, 1, qs], in0=ps1[:64, :], in1=rb[:64, :]
                            )
                        units.append(u)
                        kinds.append("av")
                for st in range(ST):
                    def u(b=b, st=st):
                        sts = slice(st * 128, st * 128 + 128)
                        xn_t = lnp.tile([128, D], F32, tag="xn")
                        nc.sync.dma_start(xn_t[:], xn_d[b, st])
                        ps = psC.tile([128, 512], F32, name="psop", tag="ot")
                        for h in range(H):
                            nc.tensor.matmul(
                                ps[:, :D],
                                lhsT=ON[b, h][:, :, sts],
                                rhs=wo_sb[:, 2 * h : 2 * h + 2, :],
                                start=(h == 0),
                                stop=False,
                                perf_mode=DR,
                            )
                        # residual add on the PE: ps += I @ xn  (fp32 matmul)
                        nc.tensor.matmul(
                            ps[:, :D],
                            lhsT=identf[:],
                            rhs=xn_t[:],
                            start=False,
                            stop=True,
                        )
                        t = lnp.tile([128, D], F32, tag="t")
                        nc.scalar.copy(out=t[:], in_=ps[:, :D])
                        stats = lnp.tile([128, 6], F32, tag="st")
                        nc.vector.bn_stats(out=stats[:], in_=t[:])
                        mv = lnp.tile([128, 2], F32, tag="mv")
                        nc.vector.bn_aggr(out=mv[:], in_=stats[:])
                        sd = lnp.tile([128, 1], F32, tag="sd")
                        nc.scalar.activation(
                            out=sd[:], in_=mv[:, 1:2], func=AF.Sqrt, bias=eps_sb[:],
                        )
                        nc.vector.reciprocal(sd[:], sd[:])
                        xv = x1n[:, b, st, :]
                        nc.vector.tensor_scalar(
                            out=xv, in0=t[:], scalar1=mv[:, 0:1], scalar2=sd[:],
                            op0=ALU.subtract, op1=ALU.mult,
                        )
                        for dc in range(DC):
                            tp = psA.tile([128, 512], BF16, name="tp", tag="p512")
                            nc.tensor.transpose(
                                tp[:, :128], x1n[:, b, st, dc * 128 : dc * 128 + 128], ident[:]
                            )
                            nc.scalar.copy(
                                out=x1T[:, b, dc, 4 + st * 128 : 4 + st * 128 + 128],
                                in_=tp[:, :128],
                            )
                    units.append(u)
                    kinds.append("op")
                return units, kinds

            def conv1_units(b, w1p, psF, hT, extra_dma=None):
                units = []
                for ft in range(FT):
                    def udma(ft=ft):
                        w1 = w1p.tile([128, K, DC, 128], BF16, name="w1", tag="w1")
                        conv1_units._w1 = w1
                        nc.sync.dma_start(w1[:], wc1_d[ft])
                        if extra_dma is not None and ft < len(extra_dma):
                            extra_dma[ft]()
                    units.append(udma)
                    for qc in range(SQ):
                        def u(b=b, ft=ft, qc=qc):
                            w1 = conv1_units._w1
                            ps = psF.tile([128, 512], F32, name="c1", tag="c1")
                            idx = 0
                            for k9 in range(K):
                                for dc in range(DC):
                                    nc.tensor.matmul(
                                        ps[:],
                                        lhsT=w1[:, k9, dc, :],
                                        rhs=x1T[:, b, dc, qc * 512 + k9 : qc * 512 + k9 + 512],
                                        start=(idx == 0),
                                        stop=(idx == K * DC - 1),
                                    )
                                    idx += 1
                            nc.scalar.activation(
                                out=hT[:, ft, 4 + qc * 512 : 4 + qc * 512 + 512],
                                in_=ps[:],
                                func=AF.Relu,
                                bias=bc1_sb[:, ft : ft + 1],
                                scale=1.0,
                            )
                        units.append(u)
                return units

            def conv2(b, psG, ln2, hT, w2):
                for st in range(ST):
                    # residual pre-product on gpsimd (off critical path)
                    tmp = ln2.tile([128, D], F32, tag="tm")
                    nc.gpsimd.tensor_mul(out=tmp[:], in0=x1n[:, b, st, :], in1=g1_sb[:])
                    ps = psG.tile([128, D], F32, name="c2", tag="c2")
                    idx = 0
                    for k9 in range(K):
                        for fc in range(FT):
                            nc.tensor.matmul(
                                ps[:],
                                lhsT=hT[:, fc, st * 128 + k9 : st * 128 + k9 + 128],
                                rhs=w2[:, k9, fc, :],
                                start=(idx == 0),
                                stop=(idx == K * FT - 1),
                            )
                            idx += 1
                    t = ln2.tile([128, D], F32, tag="t")
                    nc.vector.tensor_add(out=t[:], in0=ps[:], in1=tmp[:])
                    nc.vector.tensor_add(out=t[:], in0=t[:], in1=cb_sb[:])
                    stats = ln2.tile([128, 6], F32, tag="st")
                    nc.vector.bn_stats(out=stats[:], in_=t[:])
                    mv = ln2.tile([128, 2], F32, tag="mv")
                    nc.vector.bn_aggr(out=mv[:], in_=stats[:])
                    sd = ln2.tile([128, 1], F32, tag="sd")
                    nc.scalar.activation(
                        out=sd[:], in_=mv[:, 1:2], func=AF.Sqrt, bias=eps_sb[:],
                    )
                    nc.vector.reciprocal(sd[:], sd[:])
                    ot = ln2.tile([128, D], F32, tag="o")
                    nc.vector.tensor_scalar(
                        out=ot[:], in0=t[:], scalar1=mv[:, 0:1], scalar2=sd[:],
                        op0=ALU.subtract, op1=ALU.mult,
                    )
                    nc.vector.tensor_mul(out=ot[:], in0=ot[:], in1=g2_sb[:])
                    nc.vector.tensor_add(out=ot[:], in0=ot[:], in1=be2_sb[:])
                    nc.sync.dma_start(y_d[b, st], ot[:])

            def fill_wts(kinds, w_sc, w_av, w_op):
                return [{"sc": w_sc, "av": w_av, "op": w_op}[k] for k in kinds]

            # ---- phases 1+2 share all pools: no inter-phase PSUM barrier, so
            # phase-2 score matmuls can fill phase-1's vector-bound tail ----
            attnp = octx.enter_context(tc.tile_pool(name="attnp", bufs=1))
            expp = octx.enter_context(tc.tile_pool(name="expp", bufs=2))
            mskp = octx.enter_context(tc.tile_pool(name="mskp", bufs=3))
            lnp = octx.enter_context(tc.tile_pool(name="lnp", bufs=3))
            smal = octx.enter_context(tc.tile_pool(name="smal", bufs=2))
            psA = octx.enter_context(tc.tile_pool(name="psA", bufs=2, space="PSUM"))
            psB = octx.enter_context(tc.tile_pool(name="psB", bufs=2, space="PSUM"))
            psC = octx.enter_context(tc.tile_pool(name="psC", bufs=2, space="PSUM"))
            hT0p = ctx.enter_context(tc.tile_pool(name="hT0p", bufs=1, side="right"))
            hT0 = hT0p.tile([128, FT, SP], BF16, tag="hT0")
            nc.gpsimd.memset(hT0[:, :, 0:4], 0.0)
            nc.gpsimd.memset(hT0[:, :, 4 + S : SP], 0.0)
            w1p = ctx.enter_context(tc.tile_pool(name="w1p", bufs=2, side="right"))
            psF = ctx.enter_context(
                tc.tile_pool(name="psF", bufs=2, space="PSUM", side="right")
            )
            # HAM pre-warm: ~36 N=128 dummy matmuls during the input DMA wait
            for _ in range(36):
                ps = psA.tile([128, 512], F32, name="pswarm", tag="p512")
                nc.tensor.matmul(
                    ps[:, :128], lhsT=warm_sb[:], rhs=warm_sb[:],
                    start=True, stop=True,
                )
            for u in qkv_units(0, psA):
                u()
            # phase 1: attention(b0), qkv(b1) woven in as PE filler
            ua, kinds = attn_units(0, expp, mskp, smal, lnp, psA, psB, psC, attnp)
            ub = qkv_units(1, psA)
            weave_w(ua, ub, fill_wts(kinds, 0.15, 2.2, 3.0))
            # phase 2: attention(b1) woven with conv1(b0)
            ua, kinds = attn_units(1, expp, mskp, smal, lnp, psA, psB, psC, attnp)
            ub = conv1_units(0, w1p, psF, hT0)
            weave_w(ua, ub, fill_wts(kinds, 0.3, 3.0, 3.4))

        # ---- phase 3: conv1(b1) + w2 chunk loads, then conv2(b0) ----
        with ExitStack() as p3:
            hT1p = p3.enter_context(tc.tile_pool(name="hT1p", bufs=1))
            hT1 = hT1p.tile([128, FT, SP], BF16, tag="hT1")
            nc.gpsimd.memset(hT1[:, :, 0:4], 0.0)
            nc.gpsimd.memset(hT1[:, :, 4 + S : SP], 0.0)
            w2p = p3.enter_context(tc.tile_pool(name="w2p", bufs=1))
            w2 = w2p.tile([128, K, FT, D], BF16, tag="w2")
            psG = p3.enter_context(tc.tile_pool(name="psG", bufs=4, space="PSUM"))
            ln2 = p3.enter_context(tc.tile_pool(name="ln2", bufs=2))
            # w2 loaded in 9 per-k chunks, interleaved AFTER each w1 ft load so
            # conv1(b1) never waits behind the bulk w2 transfer
            w2dmas = [
                (lambda k=k: nc.sync.dma_start(w2[:, k, :, :], wc2_d[k]))
                for k in range(K)
            ]
            for u in conv1_units(1, w1p, psF, hT1, extra_dma=w2dmas):
                u()
            conv2(0, psG, ln2, hT0, w2)
            # ---- phase 4 ----
            conv2(1, psG, ln2, hT1, w2)


def _build():
    if "nc" not in _CACHE:
        nc = bacc.Bacc()
        _CACHE["nc"] = _emit(nc)
    return _CACHE["nc"]


def _prep_shared(Wq, bq, Wk, bk, Wv, bv, Wo, bo, Wc1, bc1, Wc2, bc2, g1, beta1, g2, beta2):
    bf = ml_dtypes.bfloat16
    f8 = ml_dtypes.float8_e4m3fn
    f32 = np.float32
    sh = {}
    for nm, W in (("wq", Wq), ("wk", Wk), ("wv", Wv)):
        sh[nm] = np.ascontiguousarray(
            (W * QKV_SC).reshape(H, DC, 128, DH).transpose(2, 0, 1, 3)
        ).astype(f8)
    wo = np.zeros((128, 4, D), dtype=f8)
    bounds = ((0, 128), (128, 192), (192, 320), (320, 384))
    for c, (r0, r1) in enumerate(bounds):
        wo[: r1 - r0, c, :] = (Wo[r0:r1] * WO_SC).astype(f8)
    wo[64, 1, :] = (bo * RES_SC).astype(f8)  # ones-row bo fold (head-0 chunk 1)
    sh["wo"] = wo
    # fold g1 into Wc1, beta1 into bc1 (padding columns use -beta1/g1)
    W1f = (Wc1 * g1[None, :, None]).astype(np.float64)
    sh["wc1"] = np.ascontiguousarray(
        W1f.reshape(K, DC, 128, FT, 128).transpose(3, 2, 0, 1, 4)
    ).astype(bf)
    bc1f = (bc1.astype(np.float64) + np.einsum("kdf,d->f", Wc1.astype(np.float64), beta1.astype(np.float64))).astype(f32)
    sh["bc1t"] = np.ascontiguousarray(bc1f.reshape(FT, 128).T)
    sh["wc2"] = np.ascontiguousarray(
        Wc2.reshape(K, FT, 128, D).transpose(0, 2, 1, 3)
    ).astype(bf)
    bqk = np.zeros((128, 2, H, 2), dtype=f32)
    for i, bb in enumerate((bq, bk)):
        for h in range(H):
            bqk[:, i, h, 0] = bb[h, :128] * QKV_SC
            bqk[:64, i, h, 1] = bb[h, 128:] * QKV_SC
    sh["bqk"] = bqk
    bvr = np.zeros((H, 208), dtype=f8)
    bvr[:, :DH] = (bv * QKV_SC).astype(f8)
    bvr[:, DH] = 1.0
    sh["bvr"] = bvr
    sh["cb"] = (beta1 + bc2).astype(f32)
    pad = np.where(np.abs(g1) > 1e-6, -beta1 / np.where(g1 == 0, 1, g1), 0.0)
    sh["padv"] = np.ascontiguousarray(
        np.broadcast_to(pad.reshape(DC, 128, 1), (DC, 128, 4))
    ).astype(bf)
    sh["g1"] = g1.astype(f32)
    sh["g2"] = g2.astype(f32)
    sh["be2"] = beta2.astype(f32)
    return sh


def run_sharded(inputs, trace=False):
    nc = _build()
    x = np.asarray(inputs["x"], dtype=np.float32)
    mask = np.asarray(inputs["mask"])
    sh = _prep_shared(
        *[np.asarray(inputs[k]) for k in (
            "Wq", "bq", "Wk", "bk", "Wv", "bv", "Wo", "bo",
            "Wc1", "bc1", "Wc2", "bc2", "g1", "beta1", "g2", "beta2",
        )]
    )
    f8 = ml_dtypes.float8_e4m3fn
    in_maps = []
    for c in range(NCORES):
        xb = x[c * NB : (c + 1) * NB]  # [NB, S, D]
        m = {}
        m["xT"] = np.ascontiguousarray(xb.transpose(0, 2, 1)).reshape(NB, DC, 128, S).astype(f8)
        m["xn"] = np.ascontiguousarray((xb * RES_SC).reshape(NB, ST, 128, D))
        mb = mask[c * NB : (c + 1) * NB]
        m["mT"] = np.ascontiguousarray(
            (~mb.transpose(0, 2, 1)).astype(f8)
        ).reshape(NB, ST, 128, S)
        m.update(sh)
        in_maps.append(m)
    res = run_bass_kernel_spmd(nc, in_maps, core_ids=list(range(NCORES)), trace=trace)
    out = np.empty((B, S, D), dtype=np.float32)
    for c in range(NCORES):
        out[c * NB : (c + 1) * NB] = res.results[c]["y"].reshape(NB, S, D)
    return out, res


def kernel(**inputs):
    out, _ = run_sharded(inputs, trace=False)
    return out


# revision 50
# speedup vs baseline: 1.0361x; 1.0361x over previous
"""FFTBlock (attention + conv-FFN transformer block) on 8 Trainium2 NeuronCores.

Data-parallel over batch: 16 batch items -> 2 per core. Each core runs the
full block (MHA + LN + conv1d-FFN + LN) on its 2 batch items.

Layout strategy (per batch item, per core):
  - Host pre-transposes x -> xT [D,S] (bf16) so QKV projections contract D on
    partitions; host pre-transposes mask -> maskT [S_k,S_q] (uint8).
  - Q,K produced transposed (QT/KT [DH,S]); V natural [S,DH] with a ones
    column appended so the A@V matmul also produces softmax denominators.
  - Scores computed transposed (scoresT [S_k,S_q]); softmax is exp-only
    (no max subtraction needed: scores are O(1); masked lanes underflow to 0
    exactly, matching the reference), denominator comes from the ones column.
  - Out-projection consumes the normalized O-transposed chunks directly and
    lands attention output in natural [S,D] layout for a free-dim layernorm.
    bo is folded into Wo via a ones row in the head-0 second chunk.
  - LN1 output is stored UN-affined (g1/beta1 folded into Wc1/bc1 with a
    -beta1/g1 padding value for 'SAME' edges); the conv2 epilogue re-applies
    g1 and (beta1+bc2) to the residual.
  - conv1 produces hT [F,S] (weights stationary), conv2 consumes hT slices as
    stationary operands and lands natural [S,D] for the second layernorm.
  - Both convs implement K=9 'same' padding via shifted slices of a
    zero-padded S axis (4+S+4 columns).
  - All weights are host-pre-arranged into their final SBUF layouts so every
    weight DMA is a dense contiguous copy.
  - ~36 dummy matmuls at t=0 pre-warm the PE HAM clock gate during input DMA.
"""

import sys

sys.path.insert(0, "/opt/trn_rl_repo")

import math
from contextlib import ExitStack

import ml_dtypes
import numpy as np

import concourse.bass as bass
import concourse.mybir as mybir
import concourse.tile as tile
from concourse import bacc
from concourse.bass_utils import run_bass_kernel_spmd
from concourse.masks import make_identity

BF16 = mybir.dt.bfloat16
F32 = mybir.dt.float32
FP8 = mybir.dt.float8e4
DR = mybir.MatmulPerfMode.DoubleRow
AF = mybir.ActivationFunctionType
ALU = mybir.AluOpType

B, S, D, H, DH, F, K = 16, 1024, 384, 2, 192, 1536, 9
NCORES = 8
NB = B // NCORES  # batch items per core
EPS = 1e-5
ISCALE = 1.0 / math.sqrt(D)  # NOTE: reference scales by sqrt(d_model)
# fp8 attention pre-scales (keep e4m3 operands in its sweet range)
QKV_SC = 64.0  # Wq/Wk/Wv/bqk/bv scale; Q,K,V carried as 64x
WO_SC = 32.0  # Wo scale; out-proj PSUM = 64*32=2048x attn; xn pre-scaled 2048x
RES_SC = QKV_SC * WO_SC
SP = S + 8  # padded sequence length (4 left, 4 right)
DC = D // 128  # 3 d-chunks
FT = F // 128  # 12 filter tiles
ST = S // 128  # 8 seq tiles of 128
SQ = S // 512  # 2 seq chunks of 512

_CACHE = {}


def _bcast(ap, p=128):
    return bass.AP(tensor=ap.tensor, offset=ap.offset, ap=[[0, p]] + list(ap.ap))


def _emit(nc):
    # ---- DRAM I/O (all weights pre-arranged host-side to final SBUF layout) ----
    xT_d = nc.dram_tensor("xT", [NB, DC, 128, S], FP8, kind="ExternalInput")
    xn_d = nc.dram_tensor("xn", [NB, ST, 128, D], F32, kind="ExternalInput")
    mT_d = nc.dram_tensor("mT", [NB, ST, 128, S], FP8, kind="ExternalInput")
    wq_d = nc.dram_tensor("wq", [128, H, DC, DH], FP8, kind="ExternalInput")
    wk_d = nc.dram_tensor("wk", [128, H, DC, DH], FP8, kind="ExternalInput")
    wv_d = nc.dram_tensor("wv", [128, H, DC, DH], FP8, kind="ExternalInput")
    wo_d = nc.dram_tensor("wo", [128, 4, D], FP8, kind="ExternalInput")
    wc1_d = nc.dram_tensor("wc1", [FT, 128, K, DC, 128], BF16, kind="ExternalInput")
    wc2_d = nc.dram_tensor("wc2", [K, 128, FT, D], BF16, kind="ExternalInput")
    bqk_d = nc.dram_tensor("bqk", [128, 2, H, 2], F32, kind="ExternalInput")
    bvr_d = nc.dram_tensor("bvr", [H, 208], FP8, kind="ExternalInput")
    bc1_d = nc.dram_tensor("bc1t", [128, FT], F32, kind="ExternalInput")
    cb_d = nc.dram_tensor("cb", [D], F32, kind="ExternalInput")
    padv_d = nc.dram_tensor("padv", [DC, 128, 4], BF16, kind="ExternalInput")
    g1_d = nc.dram_tensor("g1", [D], F32, kind="ExternalInput")
    g2_d = nc.dram_tensor("g2", [D], F32, kind="ExternalInput")
    be2_d = nc.dram_tensor("be2", [D], F32, kind="ExternalInput")
    y_d = nc.dram_tensor("y", [NB, ST, 128, D], F32, kind="ExternalOutput")

    with tile.TileContext(nc) as tc:
        _body(nc, tc, locals())
    nc.finalize()
    return nc


def _body(nc, tc, d):
    xT_d, xn_d, mT_d = d["xT_d"], d["xn_d"], d["mT_d"]
    wq_d, wk_d, wv_d, wo_d = d["wq_d"], d["wk_d"], d["wv_d"], d["wo_d"]
    wc1_d, wc2_d = d["wc1_d"], d["wc2_d"]
    bqk_d, bvr_d, bc1_d, cb_d, padv_d = (
        d["bqk_d"], d["bvr_d"], d["bc1_d"], d["cb_d"], d["padv_d"],
    )
    g1_d, g2_d, be2_d, y_d = d["g1_d"], d["g2_d"], d["be2_d"], d["y_d"]

    with ExitStack() as ctx:
        const = ctx.enter_context(tc.tile_pool(name="const", bufs=1))
        persist = ctx.enter_context(tc.tile_pool(name="persist", bufs=1))

        # ---- critical-path weights first (dense contiguous DMAs) ----
        wq_sb = const.tile([128, H, DC, DH], FP8, tag="wq")
        nc.sync.dma_start(wq_sb[:], wq_d[:])
        bqk_sb = const.tile([128, 2, H, 2], F32, tag="bqk")
        nc.sync.dma_start(bqk_sb[:], bqk_d[:])

        # warm-up operand for HAM pre-warming
        warm_sb = const.tile([128, 128], BF16, tag="warm")
        nc.gpsimd.memset(warm_sb[:], 0.0)

        # ---- phased execution ----
        # P1: attention(b0) + qkv(b1) filler   P2: attention(b1) || conv1(b0)
        # P3: conv1(b1) || [w2 load + conv2(b0)]   P4: conv2(b1)
        with ExitStack() as octx:
            qkvp = octx.enter_context(tc.tile_pool(name="qkvp", bufs=1))
            xT_sb = [
                qkvp.tile([128, DC, S], FP8, name=f"xT{b}", tag=f"xT{b}")
                for b in range(NB)
            ]
            # b0's x first (unblocks Q/K h0 ASAP), then remaining weights
            for dc in range(DC):
                nc.sync.dma_start(xT_sb[0][:, dc, :], xT_d[0, dc])
            wk_sb = const.tile([128, H, DC, DH], FP8, tag="wk")
            nc.sync.dma_start(wk_sb[:], wk_d[:])
            wv_sb = const.tile([128, H, DC, DH], FP8, tag="wv")
            nc.sync.dma_start(wv_sb[:], wv_d[:])
            bvr_sb = const.tile([128, H, 208], FP8, tag="bvr")
            nc.sync.dma_start(bvr_sb[:], _bcast(bvr_d[:]))
            for dc in range(DC):
                nc.sync.dma_start(xT_sb[1][:, dc, :], xT_d[1, dc])

            # remaining constants (off the critical path)
            wo_sb = const.tile([128, 4, D], FP8, tag="wo")
            nc.sync.dma_start(wo_sb[:], wo_d[:])
            ident = const.tile([128, 128], BF16, tag="ident")
            make_identity(nc, ident[:])
            identf = const.tile([128, 128], F32, tag="identf")
            make_identity(nc, identf[:])
            bc1_sb = const.tile([128, FT], F32, tag="bc1")
            nc.sync.dma_start(bc1_sb[:], bc1_d[:])
            cb_sb = const.tile([128, D], F32, tag="cb")
            nc.sync.dma_start(cb_sb[:], _bcast(cb_d[:]))
            g1_sb = const.tile([128, D], F32, tag="g1")
            nc.sync.dma_start(g1_sb[:], _bcast(g1_d[:]))
            g2_sb = const.tile([128, D], F32, tag="g2")
            nc.sync.dma_start(g2_sb[:], _bcast(g2_d[:]))
            be2_sb = const.tile([128, D], F32, tag="be2")
            nc.sync.dma_start(be2_sb[:], _bcast(be2_d[:]))
            eps_sb = const.tile([128, 1], F32, tag="eps")
            nc.vector.memset(eps_sb[:], EPS)

            x1T = persist.tile([128, NB, DC, SP], BF16, tag="x1T")
            x1n = persist.tile([128, NB, ST, D], BF16, tag="x1n")
            for b in range(NB):
                for dc in range(DC):
                    # 'SAME'-edge padding value -beta1/g1 (cancels the folded
                    # beta1 bias for taps that fall outside the sequence)
                    nc.sync.dma_start(x1T[:, b, dc, 0:4], padv_d[dc])
                    nc.sync.dma_start(x1T[:, b, dc, 4 + S : SP], padv_d[dc])

            QT, KT, VV, ON = {}, {}, {}, {}

            def weave_w(a, b, wts):
                # place filler units from b after a-units per-unit quota wts
                acc = 0.0
                ib = 0
                for ua, w in zip(a, wts):
                    ua()
                    acc += w
                    while ib < len(b) and acc >= 1.0:
                        b[ib]()
                        ib += 1
                        acc -= 1.0
                while ib < len(b):
                    b[ib]()
                    ib += 1

            def qkv_units(b, psA):
                units = []
                for h in range(H):
                    qt = qkvp.tile([128, 2, S], FP8, name=f"qt{b}{h}", tag=f"qt{b}{h}")
                    kt = qkvp.tile([128, 2, S], FP8, name=f"kt{b}{h}", tag=f"kt{b}{h}")
                    vv = qkvp.tile([128, ST, 208], FP8, name=f"vv{b}{h}", tag=f"vv{b}{h}")
                    QT[b, h], KT[b, h], VV[b, h] = qt, kt, vv
                    # zero the unused dh rows so DoubleRow pair-reads see 0
                    nc.gpsimd.memset(qt[64:128, 1, :], 0.0)
                    nc.gpsimd.memset(kt[64:128, 1, :], 0.0)
                    for wsb, bi, dst in ((wq_sb, 0, qt), (wk_sb, 1, kt)):
                        for mc, (m0, msz) in enumerate(((0, 128), (128, 64))):
                            for qc in range(SQ):
                                def u(b=b, h=h, wsb=wsb, bi=bi, dst=dst, m0=m0, msz=msz, mc=mc, qc=qc):
                                    qs = slice(qc * 512, qc * 512 + 512)
                                    ps = psA.tile([128, 512], F32, name="psqk", tag="p512")
                                    nc.tensor.matmul(
                                        ps[:msz, :],
                                        lhsT=wsb[:, h, 0:2, m0 : m0 + msz],
                                        rhs=xT_sb[b][:, 0:2, qs],
                                        start=True,
                                        stop=False,
                                        perf_mode=DR,
                                    )
                                    nc.tensor.matmul(
                                        ps[:msz, :],
                                        lhsT=wsb[:, h, 2, m0 : m0 + msz],
                                        rhs=xT_sb[b][:, 2, qs],
                                        start=False,
                                        stop=True,
                                    )
                                    nc.scalar.activation(
                                        out=dst[:msz, mc, qc * 512 : qc * 512 + 512],
                                        in_=ps[:msz, :],
                                        func=AF.Identity,
                                        bias=bqk_sb[:msz, bi, h, mc : mc + 1],
                                        scale=1.0,
                                    )
                                units.append(u)
                    for st in range(ST):
                        def u(b=b, h=h, vv=vv, st=st):
                            ss = slice(st * 128, st * 128 + 128)
                            ps = psA.tile([128, 512], F32, name="psv", tag="p512")
                            nc.tensor.matmul(
                                ps[:, :DH],
                                lhsT=xT_sb[b][:, 0:2, ss],
                                rhs=wv_sb[:, h, 0:2, :],
                                start=True,
                                stop=False,
                                perf_mode=DR,
                            )
                            nc.tensor.matmul(
                                ps[:, :DH],
                                lhsT=xT_sb[b][:, 2, ss],
                                rhs=wv_sb[:, h, 2, :],
                                start=False,
                                stop=True,
                            )
                            nc.vector.tensor_add(
                                out=vv[:, st, 0:DH], in0=ps[:, :DH],
                                in1=bvr_sb[:, h, 0:DH],
                            )
                            nc.gpsimd.memset(vv[:, st, DH : DH + 1], 1.0)
                        units.append(u)
                return units

            def attn_units(b, expp, mskp, smal, lnp, psA, psB, psC, attn):
                units, kinds = [], []
                expTs = {}
                for h in range(H):
                    expT = expp.tile([128, ST, S], FP8, name=f"expT{h}", tag="expT")
                    expTs[h] = expT
                    for kc in range(ST):
                        def u(b=b, h=h, expT=expT, kc=kc):
                            qt, kt = QT[b, h], KT[b, h]
                            mtile = mskp.tile([128, 1024], FP8, name="mt", tag="mt")
                            nc.sync.dma_start(mtile[:], mT_d[b, kc])
                            for qc in range(SQ):
                                qs = slice(qc * 512, qc * 512 + 512)
                                ps = psB.tile([128, 512], F32, name="pssc", tag="sc")
                                nc.tensor.matmul(
                                    ps[:],
                                    lhsT=kt[:, :, kc * 128 : kc * 128 + 128],
                                    rhs=qt[:, :, qs],
                                    start=True,
                                    stop=True,
                                    perf_mode=DR,
                                )
                                nc.scalar.activation(
                                    out=expT[:, kc, qs], in_=ps[:], func=AF.Exp,
                                    scale=ISCALE / (QKV_SC * QKV_SC),
                                )
                                nc.vector.tensor_mul(
                                    out=expT[:, kc, qs], in0=expT[:, kc, qs],
                                    in1=mtile[:, qs],
                                )
                        units.append(u)
                        kinds.append("sc")
                for h in range(H):
                    onrm = attn.tile([128, 2, S], FP8, name=f"on{b}{h}", tag=f"on{b}{h}")
                    ON[b, h] = onrm
                    nc.gpsimd.memset(onrm[64:128, 1, :], 0.0)
                    if h == 0:
                        # ones row (row 64 of chunk 1) -> adds bo via Wo's bo row
                        nc.gpsimd.memset(onrm[64:65, 1, :], 1.0)
                # alternate heads so AV matmul bursts overlap normalize chains
                for qc in range(SQ):
                    for h in range(H):
                        def u(b=b, h=h, qc=qc):
                            expT, onrm = expTs[h], ON[b, h]
                            vv = VV[b, h]
                            qs = slice(qc * 512, qc * 512 + 512)
                            ps0 = psC.tile([128, 512], F32, name="ps0", tag="ot")
                            ps1 = psC.tile([128, 512], F32, name="ps1", tag="ot")
                            for kc in range(0, ST, 2):
                                nc.tensor.matmul(
                                    ps0[:],
                                    lhsT=vv[:, kc : kc + 2, 0:128],
                                    rhs=expT[:, kc : kc + 2, qs],
                                    start=(kc == 0),
                                    stop=(kc == ST - 2),
                                    perf_mode=DR,
                                )
                                nc.tensor.matmul(
                                    ps1[:65, :],
                                    lhsT=vv[:, kc : kc + 2, 128 : DH + 1],
                                    rhs=expT[:, kc : kc + 2, qs],
                                    start=(kc == 0),
                                    stop=(kc == ST - 2),
                                    perf_mode=DR,
                                )
                            rc = smal.tile([1, 512], F32, tag="rc")
                            nc.scalar.copy(out=rc[:], in_=ps1[64:65, :])
                            rb = smal.tile([128, 512], F32, tag="rb")
                            nc.gpsimd.partition_broadcast(rb[:], rc[:])
                            nc.vector.reciprocal(rb[:], rb[:])
                            nc.vector.tensor_mul(out=onrm[:, 0, qs], in0=ps0[:], in1=rb[:])
                            nc.vector.tensor_mul(
                                out=onrm[:64, 1, qs], in0=ps1[:64, :], in1=rb[:64, :]
                            )
                        units.append(u)
                        kinds.append("av")
                for st in range(ST):
                    def u(b=b, st=st):
                        sts = slice(st * 128, st * 128 + 128)
                        xn_t = lnp.tile([128, D], F32, tag="xn")
                        nc.sync.dma_start(xn_t[:], xn_d[b, st])
                        ps = psC.tile([128, 512], F32, name="psop", tag="ot")
                        for h in range(H):
                            nc.tensor.matmul(
                                ps[:, :D],
                                lhsT=ON[b, h][:, :, sts],
                                rhs=wo_sb[:, 2 * h : 2 * h + 2, :],
                                start=(h == 0),
                                stop=(h == 1),
                                perf_mode=DR,
                            )
                        t = lnp.tile([128, D], F32, tag="t")
                        nc.vector.tensor_add(out=t[:], in0=ps[:, :D], in1=xn_t[:])
                        stats = lnp.tile([128, 6], F32, tag="st")
                        nc.vector.bn_stats(out=stats[:], in_=t[:])
                        mv = lnp.tile([128, 2], F32, tag="mv")
                        nc.vector.bn_aggr(out=mv[:], in_=stats[:])
                        sd = lnp.tile([128, 1], F32, tag="sd")
                        nc.scalar.activation(
                            out=sd[:], in_=mv[:, 1:2], func=AF.Sqrt, bias=eps_sb[:],
                        )
                        nc.vector.reciprocal(sd[:], sd[:])
                        xv = x1n[:, b, st, :]
                        nc.vector.tensor_scalar(
                            out=xv, in0=t[:], scalar1=mv[:, 0:1], scalar2=sd[:],
                            op0=ALU.subtract, op1=ALU.mult,
                        )
                        for dc in range(DC):
                            tp = psA.tile([128, 512], BF16, name="tp", tag="p512")
                            nc.tensor.transpose(
                                tp[:, :128], x1n[:, b, st, dc * 128 : dc * 128 + 128], ident[:]
                            )
                            nc.scalar.copy(
                                out=x1T[:, b, dc, 4 + st * 128 : 4 + st * 128 + 128],
                                in_=tp[:, :128],
                            )
                    units.append(u)
                    kinds.append("op")
                return units, kinds

            def conv1_units(b, w1p, psF, hT, extra_dma=None):
                units = []
                for ft in range(FT):
                    def udma(ft=ft):
                        w1 = w1p.tile([128, K, DC, 128], BF16, name="w1", tag="w1")
                        conv1_units._w1 = w1
                        nc.sync.dma_start(w1[:], wc1_d[ft])
                        if extra_dma is not None and ft < len(extra_dma):
                            extra_dma[ft]()
                    units.append(udma)
                    for qc in range(SQ):
                        def u(b=b, ft=ft, qc=qc):
                            w1 = conv1_units._w1
                            ps = psF.tile([128, 512], F32, name="c1", tag="c1")
                            idx = 0
                            for k9 in range(K):
                                for dc in range(DC):
                                    nc.tensor.matmul(
                                        ps[:],
                                        lhsT=w1[:, k9, dc, :],
                                        rhs=x1T[:, b, dc, qc * 512 + k9 : qc * 512 + k9 + 512],
                                        start=(idx == 0),
                                        stop=(idx == K * DC - 1),
                                    )
                                    idx += 1
                            nc.scalar.activation(
                                out=hT[:, ft, 4 + qc * 512 : 4 + qc * 512 + 512],
                                in_=ps[:],
                                func=AF.Relu,
                                bias=bc1_sb[:, ft : ft + 1],
                                scale=1.0,
                            )
                        units.append(u)
                return units

            def conv2(b, psG, ln2, hT, w2):
                for st in range(ST):
                    # residual pre-product on gpsimd (off critical path)
                    tmp = ln2.tile([128, D], F32, tag="tm")
                    nc.gpsimd.tensor_mul(out=tmp[:], in0=x1n[:, b, st, :], in1=g1_sb[:])
                    ps = psG.tile([128, D], F32, name="c2", tag="c2")
                    idx = 0
                    for k9 in range(K):
                        for fc in range(FT):
                            nc.tensor.matmul(
                                ps[:],
                                lhsT=hT[:, fc, st * 128 + k9 : st * 128 + k9 + 128],
                                rhs=w2[:, k9, fc, :],
                                start=(idx == 0),
                                stop=(idx == K * FT - 1),
                            )
                            idx += 1
                    t = ln2.tile([128, D], F32, tag="t")
                    nc.vector.tensor_add(out=t[:], in0=ps[:], in1=tmp[:])
                    nc.vector.tensor_add(out=t[:], in0=t[:], in1=cb_sb[:])
                    stats = ln2.tile([128, 6], F32, tag="st")
                    nc.vector.bn_stats(out=stats[:], in_=t[:])
                    mv = ln2.tile([128, 2], F32, tag="mv")
                    nc.vector.bn_aggr(out=mv[:], in_=stats[:])
                    sd = ln2.tile([128, 1], F32, tag="sd")
                    nc.scalar.activation(
                        out=sd[:], in_=mv[:, 1:2], func=AF.Sqrt, bias=eps_sb[:],
                    )
                    nc.vector.reciprocal(sd[:], sd[:])
                    ot = ln2.tile([128, D], F32, tag="o")
                    nc.vector.tensor_scalar(
                        out=ot[:], in0=t[:], scalar1=mv[:, 0:1], scalar2=sd[:],
                        op0=ALU.subtract, op1=ALU.mult,
                    )
                    nc.vector.tensor_mul(out=ot[:], in0=ot[:], in1=g2_sb[:])
                    nc.vector.tensor_add(out=ot[:], in0=ot[:], in1=be2_sb[:])
                    nc.sync.dma_start(y_d[b, st], ot[:])

            def fill_wts(kinds, w_sc, w_av, w_op):
                return [{"sc": w_sc, "av": w_av, "op": w_op}[k] for k in kinds]

            # ---- phases 1+2 share all pools: no inter-phase PSUM barrier, so
            # phase-2 score matmuls can fill phase-1's vector-bound tail ----
            attnp = octx.enter_context(tc.tile_pool(name="attnp", bufs=1))
            expp = octx.enter_context(tc.tile_pool(name="expp", bufs=2))
            mskp = octx.enter_context(tc.tile_pool(name="mskp", bufs=3))
            lnp = octx.enter_context(tc.tile_pool(name="lnp", bufs=3))
            smal = octx.enter_context(tc.tile_pool(name="smal", bufs=2))
            psA = octx.enter_context(tc.tile_pool(name="psA", bufs=2, space="PSUM"))
            psB = octx.enter_context(tc.tile_pool(name="psB", bufs=2, space="PSUM"))
            psC = octx.enter_context(tc.tile_pool(name="psC", bufs=2, space="PSUM"))
            hT0p = ctx.enter_context(tc.tile_pool(name="hT0p", bufs=1, side="right"))
            hT0 = hT0p.tile([128, FT, SP], BF16, tag="hT0")
            nc.gpsimd.memset(hT0[:, :, 0:4], 0.0)
            nc.gpsimd.memset(hT0[:, :, 4 + S : SP], 0.0)
            w1p = ctx.enter_context(tc.tile_pool(name="w1p", bufs=2, side="right"))
            psF = ctx.enter_context(
                tc.tile_pool(name="psF", bufs=2, space="PSUM", side="right")
            )
            # HAM pre-warm: ~36 N=128 dummy matmuls during the input DMA wait
            for _ in range(36):
                ps = psA.tile([128, 512], F32, name="pswarm", tag="p512")
                nc.tensor.matmul(
                    ps[:, :128], lhsT=warm_sb[:], rhs=warm_sb[:],
                    start=True, stop=True,
                )
            for u in qkv_units(0, psA):
                u()
            # phase 1: attention(b0), qkv(b1) woven in as PE filler
            ua, kinds = attn_units(0, expp, mskp, smal, lnp, psA, psB, psC, attnp)
            ub = qkv_units(1, psA)
            weave_w(ua, ub, fill_wts(kinds, 0.15, 2.2, 3.0))
            # phase 2: attention(b1) woven with conv1(b0)
            ua, kinds = attn_units(1, expp, mskp, smal, lnp, psA, psB, psC, attnp)
            ub = conv1_units(0, w1p, psF, hT0)
            weave_w(ua, ub, fill_wts(kinds, 0.3, 3.0, 3.4))

        # ---- phase 3: conv1(b1) + w2 chunk loads, then conv2(b0) ----
        with ExitStack() as p3:
            hT1p = p3.enter_context(tc.tile_pool(name="hT1p", bufs=1))
            hT1 = hT1p.tile([128, FT, SP], BF16, tag="hT1")
            nc.gpsimd.memset(hT1[:, :, 0:4], 0.0)
            nc.gpsimd.memset(hT1[:, :, 4 + S : SP], 0.0)
            w2p = p3.enter_context(tc.tile_pool(name="w2p", bufs=1))
            w2 = w2p.tile([128, K, FT, D], BF16, tag="w2")
            psG = p3.enter_context(tc.tile_pool(name="psG", bufs=4, space="PSUM"))
            ln2 = p3.enter_context(tc.tile_pool(name="ln2", bufs=2))
            # w2 loaded in 9 per-k chunks, interleaved AFTER each w1 ft load so
            # conv1(b1) never waits behind the bulk w2 transfer
            w2dmas = [
                (lambda k=k: nc.sync.dma_start(w2[:, k, :, :], wc2_d[k]))
                for k in range(K)
            ]
            for u in conv1_units(1, w1p, psF, hT1, extra_dma=w2dmas):
                u()
            conv2(0, psG, ln2, hT0, w2)
            # ---- phase 4 ----
            conv2(1, psG, ln2, hT1, w2)


def _build():
    if "nc" not in _CACHE:
        nc = bacc.Bacc()
        _CACHE["nc"] = _emit(nc)
    return _CACHE["nc"]


def _prep_shared(Wq, bq, Wk, bk, Wv, bv, Wo, bo, Wc1, bc1, Wc2, bc2, g1, beta1, g2, beta2):
    bf = ml_dtypes.bfloat16
    f8 = ml_dtypes.float8_e4m3fn
    f32 = np.float32
    sh = {}
    for nm, W in (("wq", Wq), ("wk", Wk), ("wv", Wv)):
        sh[nm] = np.ascontiguousarray(
            (W * QKV_SC).reshape(H, DC, 128, DH).transpose(2, 0, 1, 3)
        ).astype(f8)
    wo = np.zeros((128, 4, D), dtype=f8)
    bounds = ((0, 128), (128, 192), (192, 320), (320, 384))
    for c, (r0, r1) in enumerate(bounds):
        wo[: r1 - r0, c, :] = (Wo[r0:r1] * WO_SC).astype(f8)
    wo[64, 1, :] = (bo * RES_SC).astype(f8)  # ones-row bo fold (head-0 chunk 1)
    sh["wo"] = wo
    # fold g1 into Wc1, beta1 into bc1 (padding columns use -beta1/g1)
    W1f = (Wc1 * g1[None, :, None]).astype(np.float64)
    sh["wc1"] = np.ascontiguousarray(
        W1f.reshape(K, DC, 128, FT, 128).transpose(3, 2, 0, 1, 4)
    ).astype(bf)
    bc1f = (bc1.astype(np.float64) + np.einsum("kdf,d->f", Wc1.astype(np.float64), beta1.astype(np.float64))).astype(f32)
    sh["bc1t"] = np.ascontiguousarray(bc1f.reshape(FT, 128).T)
    sh["wc2"] = np.ascontiguousarray(
        Wc2.reshape(K, FT, 128, D).transpose(0, 2, 1, 3)
    ).astype(bf)
    bqk = np.zeros((128, 2, H, 2), dtype=f32)
    for i, bb in enumerate((bq, bk)):
        for h in range(H):
            bqk[:, i, h, 0] = bb[h, :128] * QKV_SC
            bqk[:64, i, h, 1] = bb[h, 128:] * QKV_SC
    sh["bqk"] = bqk
    bvr = np.zeros((H, 208), dtype=f8)
    bvr[:, :DH] = (bv * QKV_SC).astype(f8)
    bvr[:, DH] = 1.0
    sh["bvr"] = bvr
    sh["cb"] = (beta1 + bc2).astype(f32)
    pad = np.where(np.abs(g1) > 1e-6, -beta1 / np.where(g1 == 0, 1, g1), 0.0)
    sh["padv"] = np.ascontiguousarray(
        np.broadcast_to(pad.reshape(DC, 128, 1), (DC, 128, 4))
    ).astype(bf)
    sh["g1"] = g1.astype(f32)
    sh["g2"] = g2.astype(f32)
    sh["be2"] = beta2.astype(f32)
    return sh


def run_sharded(inputs, trace=False):
    nc = _build()
    x = np.asarray(inputs["x"], dtype=np.float32)
    mask = np.asarray(inputs["mask"])
    sh = _prep_shared(
        *[np.asarray(inputs[k]) for k in (
            "Wq", "bq", "Wk", "bk", "Wv", "bv", "Wo", "bo",
            "Wc1", "bc1", "Wc2", "bc2", "g1", "beta1", "g2", "beta2",
        )]
    )
    f8 = ml_dtypes.float8_e4m3fn
    in_maps = []
    for c in range(NCORES):
        xb = x[c * NB : (c + 1) * NB]  # [NB, S, D]
        m = {}
        m["xT"] = np.ascontiguousarray(xb.transpose(0, 2, 1)).reshape(NB, DC, 128, S).astype(f8)
        m["xn"] = np.ascontiguousarray((xb * RES_SC).reshape(NB, ST, 128, D))
        mb = mask[c * NB : (c + 1) * NB]
        m["mT"] = np.ascontiguousarray(
            (~mb.transpose(0, 2, 1)).astype(f8)
        ).reshape(NB, ST, 128, S)
        m.update(sh)
        in_maps.append(m)
    res = run_bass_kernel_spmd(nc, in_maps, core_ids=list(range(NCORES)), trace=trace)
    out = np.empty((B, S, D), dtype=np.float32)
    for c in range(NCORES):
        out[c * NB : (c + 1) * NB] = res.results[c]["y"].reshape(NB, S, D)
    return out, res


def kernel(**inputs):
    out, _ = run_sharded(inputs, trace=False)
    return out
